# revision 1
# baseline (speedup 1.0000x reference)
"""W4A4 quantized linear (AutoQVLALinearW4A4) on 8 Trainium2 NeuronCores.

y = dequant_rowwise_quant(x) @ dequant_w4(qweight)^T + bias

Tensor-parallel over out_features: each core gets a 512-row slice of the
packed weights / scales / bias, the full x (row-reversed), and produces a
[4096, 512] slice of the output. The per-token amax work is token-sharded
and shared via a tiny AllGather.

Device algorithm (per core), exact-integer math on the PE:
  1. amax for OWN 512 tokens (int16 abs-bit trick + max tree on DVE),
     a_scale = max/7, AllGather of all cores' a_scales (2KB).
  2. qb = fp16(x * (1/a_scale) + 1536)  (exact round-half-even to int+1536)
     q8 = fp8_e4m3(qb - 1536)           (ACT pass, exact ints in [-8,7])
  3. transpose q8 via DMA-transpose of byte-PAIRS viewed as fp16: partition
     jj of chunk c holds bytes (k=256c+2jj, k=256c+2jj+1).
  4. fp8 DoubleRowSwInterleave matmuls: q^T byte-pairs are the STATIONARY
     operand (SwInterleave wants interleaved pairs; its column reversal is
     cancelled by feeding x row-reversed from the host), unpacked int4
     weights are the MOVING operand as two separated k-planes.
     Output lands directly as [token, n] in PSUM, exact ints in fp32.
  5. epilogue: (psum * a_scale_pp) * wscale_bcast + bias_bcast on DVE.
     a_scale needs a partition flip (out tokens run opposite to the
     x_rev rows) -> one tiny anti-diagonal matmul (J @ s).
"""

import numpy as np
import concourse.bass as bass
import concourse.mybir as mybir
from concourse import bacc
from concourse.tile import TileContext
from concourse.bass_utils import run_bass_kernel_spmd

F8 = mybir.dt.float8e4
F16 = mybir.dt.float16
F32 = mybir.dt.float32
I8 = mybir.dt.int8
I16 = mybir.dt.int16
AOP = mybir.AluOpType
ACTF = mybir.ActivationFunctionType
SWI = mybir.MatmulPerfMode.DoubleRowSwInterleave

N_CORES = 8


def build(M=4096, K=4096, NS=512, use_cc=True, mm_bufs=7, qt_bufs=8,
          repeat=1):
    """Build + compile the per-core program. Returns the Bacc object."""
    assert M % (128 * N_CORES) == 0 and K % 256 == 0 and NS % 128 == 0
    T = M // 128          # token tiles
    NT = NS // 128        # out-feature subtiles
    C = K // 256          # DoubleRow contraction chunks
    KP = K // 2           # packed weight columns
    TO = T // N_CORES     # own token tiles (for amax sharding)

    nc = bacc.Bacc("TRN2", target_bir_lowering=False, debug=False,
                   num_devices=N_CORES)

    x_d = nc.dram_tensor("x", [M, K], F16, kind="ExternalInput")  # reversed!
    xo_d = nc.dram_tensor("xown", [TO * 128, K], F16, kind="ExternalInput")
    wp_d = nc.dram_tensor("wp", [NS, KP], I8, kind="ExternalInput")
    ws_d = nc.dram_tensor("wsc", [1, NS], F16, kind="ExternalInput")
    b_d = nc.dram_tensor("bias", [1, NS], F16, kind="ExternalInput")
    y_d = nc.dram_tensor("y", [M, NS], F16, kind="ExternalOutput")
    if use_cc:
        cc_in = nc.dram_tensor("cc_in", [1, TO * 128], F32)
        cc_out = nc.dram_tensor("cc_out", [N_CORES, TO * 128], F32,
                                addr_space="Shared")

    with TileContext(nc) as tc:
        with (
            tc.tile_pool(name="const", bufs=1) as cpool,
            tc.tile_pool(name="wsetup", bufs=2) as wpool,
            tc.tile_pool(name="xwork", bufs=3) as xpool,
            tc.tile_pool(name="qtp", bufs=qt_bufs) as qpool,
            tc.tile_pool(name="small", bufs=3) as spool,
            tc.tile_pool(name="epi", bufs=4) as epool,
            tc.tile_pool(name="psum", bufs=mm_bufs, space="PSUM") as ppool,
        ):
            # ---------------- constants ----------------
            wsc_row = cpool.tile([1, NS], F16)
            nc.sync.dma_start(wsc_row[:, :], ws_d.ap())
            wsc_bc = cpool.tile([128, NS], F16)
            nc.gpsimd.partition_broadcast(wsc_bc[:, :], wsc_row[:, :])
            bias_row = cpool.tile([1, NS], F16)
            nc.sync.dma_start(bias_row[:, :], b_d.ap())
            bias_bc = cpool.tile([128, NS], F16)
            nc.gpsimd.partition_broadcast(bias_bc[:, :], bias_row[:, :])
            # anti-diagonal J for the partition flip
            jm = cpool.tile([128, 128], F32)
            nc.vector.memset(jm[:, :], 1.0)
            nc.gpsimd.affine_select(jm[:, :], jm[:, :], pattern=[[1, 128]],
                                    base=-127, channel_multiplier=1,
                                    compare_op=AOP.is_equal, fill=0.0)

            # ---------------- x prefetch (overlap with setup) ----------
            pre_x = {}
            for i in range(4):
                xt = xpool.tile([128, K], F16, tag="x", bufs=8,
                                name=f"xt_0_{i}")
                nc.sync.dma_start(xt[:, :], x_d[i * 128:(i + 1) * 128, :])
                pre_x[i] = xt

            # ---------------- weight setup ----------------
            # wt_pairs fake-fp16 [jj, c, nt, nn]: pair (k=256c+2jj, +1) of
            # W[nt*128+nn, .]; then deinterleave into wt_sep[jj, c, i, n].
            wt_pairs = cpool.tile([128, C, NT, 128], F16)
            wt_sep = cpool.tile([128, C, 2, NS], F8)
            for nt in range(NT):
                wp_sb = wpool.tile([128, KP], I8, tag="wp")
                nc.sync.dma_start(wp_sb[:, :],
                                  wp_d[nt * 128:(nt + 1) * 128, :])
                w8 = wpool.tile([128, K], F8, tag="w8")
                w8v = w8[:, :].rearrange("p (j two) -> p j two", two=2)
                # high nibble = floor(b/16) (already sign-extended):
                # fp16(b/16 + 1535.53125) - 1536 via exact magic rounding
                hb = wpool.tile([128, KP], F16, tag="hb")
                nc.scalar.activation(hb[:, :], wp_sb[:, :], ACTF.Copy,
                                     bias=1535.53125, scale=1.0 / 16)
                nc.scalar.activation(w8v[:, :, 1], hb[:, :], ACTF.Copy,
                                     bias=-1536.0, scale=1.0)
                # low nibble: ((b & 15) ^ 8) - 8
                lo4 = wpool.tile([128, KP], I8, tag="lo4")
                nc.vector.tensor_scalar(lo4[:, :], wp_sb[:, :], 15, 8,
                                        op0=AOP.bitwise_and,
                                        op1=AOP.bitwise_xor)
                nc.vector.tensor_scalar(w8v[:, :, 0], lo4[:, :], 8.0, None,
                                        op0=AOP.subtract)
                nc.sync.dma_start_transpose(wt_pairs[:, :, nt, :],
                                            w8[:, :].bitcast(F16))
            wtp8 = wt_pairs[:, :, :, :].bitcast(F8)  # [128, C, NT, 256]
            for nt in range(NT):
                for i in range(2):
                    nc.scalar.copy(
                        wt_sep[:, :, i, nt * 128:(nt + 1) * 128],
                        wtp8[:, :, nt, :].rearrange(
                            "p c (n two) -> p c two n", two=2)[:, :, i, :])

            # ---------------- phase A: own-token amax ----------------
            s_own = spool.tile([128, TO], F32, tag="sown", bufs=1)
            for j in range(TO):
                xt = xpool.tile([128, K], F16, tag="x", bufs=8,
                                name=f"xta_{j}")
                nc.sync.dma_start(xt[:, :], xo_d[j * 128:(j + 1) * 128, :])
                xa = xpool.tile([128, K], I16, tag="xa", name=f"xa_{j}")
                nc.vector.tensor_scalar(xa[:, :], xt[:, :].bitcast(I16),
                                        0x7FFF, None, op0=AOP.bitwise_and)
                w = K // 2
                while w >= 512:
                    nc.vector.tensor_tensor(xa[:, :w], xa[:, :w],
                                            xa[:, w:2 * w], op=AOP.max)
                    w //= 2
                mbits = spool.tile([128, 1], I16, tag="mbits")
                nc.vector.tensor_reduce(mbits[:, :], xa[:, :2 * w],
                                        axis=mybir.AxisListType.X,
                                        op=AOP.max)
                nc.vector.tensor_scalar(s_own[:, j:j + 1],
                                        mbits[:, :].bitcast(F16),
                                        1e-6, 1.0 / 7.0,
                                        op0=AOP.max, op1=AOP.mult)

            # share scales (in x_rev row order) across cores
            s_rev = cpool.tile([128, T], F32)
            if use_cc:
                nc.sync.dma_start(
                    cc_in.ap().rearrange("o (j p) -> o p j", p=128),
                    s_own[:, :])
                nc.gpsimd.collective_compute(
                    "AllGather", AOP.bypass,
                    replica_groups=[list(range(N_CORES))],
                    ins=[cc_in.ap()], outs=[cc_out.ap()])
                nc.sync.dma_start(
                    s_rev[:, :],
                    cc_out.ap().rearrange("r (j p) -> p (r j)", p=128))
            else:
                nc.vector.tensor_copy(s_rev[:, :TO], s_own[:, :])

            # flipped scales for the epilogue + reciprocal for quantization
            sq_all = cpool.tile([128, T], F32)
            nc.vector.reciprocal(sq_all[:, :], s_rev[:, :])
            ps_j = ppool.tile([128, T], F32, tag="psj", bufs=1)
            nc.tensor.matmul(ps_j[:, :], jm[:, :], s_rev[:, :],
                             start=True, stop=True)
            s_flip = cpool.tile([128, T], F32)
            nc.vector.tensor_copy(s_flip[:, :], ps_j[:, :])

            # ---------------- main loop ----------------
            for rep in range(repeat):
              for i in range(T):
                  if rep == 0 and i in pre_x:
                      xt = pre_x.pop(i)
                  else:
                      xt = xpool.tile([128, K], F16, tag="x", bufs=8,
                                      name=f"xt_{rep}_{i}")
                      nc.sync.dma_start(xt[:, :],
                                        x_d[i * 128:(i + 1) * 128, :])
                  # qb = fp16(x*sq + 1536): exact RNE integer round
                  nc.vector.tensor_scalar(xt[:, :], xt[:, :],
                                          sq_all[:, i:i + 1], 1536.0,
                                          op0=AOP.mult, op1=AOP.add)
                  # q8 = fp8(qb - 1536), alternating ACT/DVE
                  q8 = xpool.tile([128, K], F8, tag="q8", name=f"q8_{rep}_{i}")
                  if i % 3 != 2:
                      nc.scalar.activation(q8[:, :], xt[:, :], ACTF.Copy,
                                           bias=-1536.0, scale=1.0)
                  else:
                      nc.vector.tensor_scalar(q8[:, :], xt[:, :], 1536.0,
                                              None, op0=AOP.subtract)
                  # pair-transpose: qT[jj, c, f] = (q[f, 256c+2jj], +1)
                  qT = qpool.tile([128, C, 128], F16, tag="qT",
                                  name=f"qT_{rep}_{i}")
                  nc.scalar.dma_start_transpose(qT[:, :, :],
                                                q8[:, :].bitcast(F16))
                  qT8 = qT[:, :, :].bitcast(F8)  # [128, C, 256]

                  ps = ppool.tile([128, NS], F32, tag="mm", name=f"ps_{rep}_{i}")
                  for c in range(C):
                      nc.tensor.matmul(ps[:, :], qT8[:, c, :],
                                       wt_sep[:, c, :, :],
                                       start=(c == 0), stop=(c == C - 1),
                                       perf_mode=SWI)
                  # epilogue: y = (ps * a_scale) * wscale + bias
                  t1 = epool.tile([128, NS], F16, tag="t1", name=f"t1_{rep}_{i}")
                  nc.vector.scalar_tensor_tensor(
                      t1[:, :], ps[:, :], s_flip[:, i:i + 1], wsc_bc[:, :],
                      op0=AOP.mult, op1=AOP.mult)
                  yout = epool.tile([128, NS], F16, tag="yo",
                                    name=f"yo_{rep}_{i}")
                  nc.vector.tensor_tensor(yout[:, :], t1[:, :], bias_bc[:, :],
                                          op=AOP.add)
                  nc.scalar.dma_start(
                      y_d[M - 128 * (i + 1):M - 128 * i, :], yout[:, :])

    nc.compile()
    return nc


_CACHE = {}


def _get_nc():
    if "nc" not in _CACHE:
        _CACHE["nc"] = build()
    return _CACHE["nc"]


def _in_maps(x, qweight_packed, w_scales, bias):
    M, K, N = 4096, 4096, 4096
    NS = N // N_CORES
    MO = M // N_CORES
    x2 = np.asarray(x).reshape(M, K)
    x_rev = np.ascontiguousarray(x2[::-1])
    wsc = np.asarray(w_scales).reshape(N)
    bias = np.asarray(bias).reshape(N)
    in_maps = []
    for c in range(N_CORES):
        sl = slice(c * NS, (c + 1) * NS)
        in_maps.append({
            "x": x_rev,
            "xown": np.ascontiguousarray(x_rev[c * MO:(c + 1) * MO]),
            "wp": np.ascontiguousarray(np.asarray(qweight_packed)[sl]),
            "wsc": np.ascontiguousarray(wsc[sl]).reshape(1, NS),
            "bias": np.ascontiguousarray(bias[sl]).reshape(1, NS),
        })
    return in_maps


def run_traced(x, qweight_packed, w_scales, bias, tmpdir=None):
    nc = _get_nc()
    in_maps = _in_maps(x, qweight_packed, w_scales, bias)
    return run_bass_kernel_spmd(nc, in_maps, core_ids=list(range(N_CORES)),
                                trace=True, tmpdir=tmpdir)


def kernel(x, qweight_packed, w_scales, bias):
    M, K, N = 4096, 4096, 4096
    NS = N // N_CORES
    MO = M // N_CORES
    nc = _get_nc()
    in_maps = _in_maps(x, qweight_packed, w_scales, bias)
    res = run_bass_kernel_spmd(nc, in_maps, core_ids=list(range(N_CORES)))
    y = np.concatenate([res.results[c]["y"] for c in range(N_CORES)], axis=1)
    return y.reshape(2, 2048, N)



# revision 2
# speedup vs baseline: 2.1816x; 2.1816x over previous
"""W4A4 quantized linear on 8 Trainium2 cores — v2: 4x token x 2x out-feature
sharding (no collectives), LDW-pipelined fp8 SwInterleave matmuls.

Per core: own 1024 tokens (row-reversed), 2048-row weight slice.
  y_blk = dequant_rowwise_quant(x_blk) @ dequant_w4(W_blk)^T + bias_blk

Device algorithm (per core), exact-integer math on the PE:
  setup: unpack W slice to fp8 k-planes (wt_sep[jj, c, nb, i, n]), amax of
         own tokens -> a_scales (+ J-matmul partition flip for the epilogue).
  per rep, per token tile i (8 tiles):
    qb = fp16(x * (1/a_scale) + 1536); q8 = fp8(qb - 1536)   (exact ints)
    qT = byte-pair DMA transpose (SwInterleave stationary format)
    for c in 16: for nb in 4: matmul(ps[:, nb], qT[c], wt_sep[c, nb], SWI)
    y = (ps * a_scale_flip) * wscale + bias  -> DMA out (row-reversed)

PE work: 8 tiles x 64 MMs x 512 cols = 109 us/core; everything else
(~21 MB DMA, ~25 us DVE, ~30 us ACT per rep) hides under it.
"""

import numpy as np
import concourse.bass as bass
import concourse.mybir as mybir
from concourse import bacc
from concourse.tile import TileContext
from concourse.bass_utils import run_bass_kernel_spmd

F8 = mybir.dt.float8e4
F16 = mybir.dt.float16
F32 = mybir.dt.float32
I8 = mybir.dt.int8
I16 = mybir.dt.int16
AOP = mybir.AluOpType
ACTF = mybir.ActivationFunctionType
SWI = mybir.MatmulPerfMode.DoubleRowSwInterleave

N_CORES = 8
TSH = 4            # token shards
NSH = 2            # out-feature shards
M, K, N = 4096, 4096, 4096
MO = M // TSH      # tokens per core (1024)
NS = N // NSH      # out features per core (2048)


def build(repeat=1, x_bufs=4, qt_bufs=3, cast_mod=2, mm_bufs=3):
    T = MO // 128      # token tiles per core (8)
    NT = NS // 128     # weight row blocks (16)
    NB = NS // 512     # 512-col output blocks (4)
    C = K // 256       # contraction chunks (16)
    KP = K // 2        # packed weight columns
    NPB = NT // NB     # 128-row blocks per 512-col block (4)

    nc = bacc.Bacc("TRN2", target_bir_lowering=False, debug=False,
                   num_devices=N_CORES)

    x_d = nc.dram_tensor("x", [MO, K], F16, kind="ExternalInput")  # reversed!
    wp_d = nc.dram_tensor("wp", [NS, KP], I8, kind="ExternalInput")
    ws_d = nc.dram_tensor("wsc", [1, NS], F16, kind="ExternalInput")
    b_d = nc.dram_tensor("bias", [1, NS], F16, kind="ExternalInput")
    y_d = nc.dram_tensor("y", [MO, NS], F16, kind="ExternalOutput")

    with TileContext(nc) as tc:
        with (
            tc.tile_pool(name="const", bufs=1) as cpool,
            tc.tile_pool(name="wsetup", bufs=2) as wpool,
            tc.tile_pool(name="xwork", bufs=2) as xpool,
            tc.tile_pool(name="qtp", bufs=qt_bufs) as qpool,
            tc.tile_pool(name="small", bufs=3) as spool,
            tc.tile_pool(name="epi", bufs=2) as epool,
            tc.tile_pool(name="psum", bufs=mm_bufs, space="PSUM") as ppool,
        ):
            # ---------------- constants ----------------
            wsc_row = cpool.tile([1, NS], F16)
            nc.sync.dma_start(wsc_row[:, :], ws_d.ap())
            wsc_bc = cpool.tile([128, NS], F16)
            nc.gpsimd.partition_broadcast(wsc_bc[:, :], wsc_row[:, :])
            bias_row = cpool.tile([1, NS], F16)
            nc.sync.dma_start(bias_row[:, :], b_d.ap())
            bias_bc = cpool.tile([128, NS], F16)
            nc.gpsimd.partition_broadcast(bias_bc[:, :], bias_row[:, :])
            # anti-diagonal J for the partition flip
            jm = cpool.tile([128, 128], F32)
            nc.vector.memset(jm[:, :], 1.0)
            nc.gpsimd.affine_select(jm[:, :], jm[:, :], pattern=[[1, 128]],
                                    base=-127, channel_multiplier=1,
                                    compare_op=AOP.is_equal, fill=0.0)

            # ---------------- x prefetch (overlap with setup) ----------
            pre_x = {}
            for i in range(min(x_bufs, T)):
                xt = xpool.tile([128, K], F16, tag="x", bufs=x_bufs,
                                name=f"xt_0_{i}")
                nc.sync.dma_start(xt[:, :], x_d[i * 128:(i + 1) * 128, :])
                pre_x[i] = xt

            # ---------------- weight setup ----------------
            # wt_sep[jj, c, nb, i, n]: fp8 W[nb*512+n, 256c+2jj+i], each
            # (c, nb) slice is a contiguous [128, 2, 512] moving operand.
            wt_sep = cpool.tile([128, C, NB, 2, 512], F8)
            for nt in range(NT):
                wp_sb = wpool.tile([128, KP], I8, tag="wp")
                nc.sync.dma_start(wp_sb[:, :],
                                  wp_d[nt * 128:(nt + 1) * 128, :])
                w8 = wpool.tile([128, K], F8, tag="w8")
                w8v = w8[:, :].rearrange("p (j two) -> p j two", two=2)
                # high nibble = floor(b/16) (already sign-extended):
                # fp16(b/16 + 1535.53125) - 1536 via exact magic rounding
                hb = wpool.tile([128, KP], F16, tag="hb")
                nc.scalar.activation(hb[:, :], wp_sb[:, :], ACTF.Copy,
                                     bias=1535.53125, scale=1.0 / 16)
                nc.scalar.activation(w8v[:, :, 1], hb[:, :], ACTF.Copy,
                                     bias=-1536.0, scale=1.0)
                # low nibble: ((b & 15) ^ 8) - 8
                lo4 = wpool.tile([128, KP], I8, tag="lo4")
                nc.vector.tensor_scalar(lo4[:, :], wp_sb[:, :], 15, 8,
                                        op0=AOP.bitwise_and,
                                        op1=AOP.bitwise_xor)
                nc.vector.tensor_scalar(w8v[:, :, 0], lo4[:, :], 8.0, None,
                                        op0=AOP.subtract)
                # byte-pair transpose: [n-row, k] -> [k-pair jj, c, n]
                wt_pairs = wpool.tile([128, C, 128], F16, tag="wtp")
                nc.sync.dma_start_transpose(wt_pairs[:, :, :],
                                            w8[:, :].bitcast(F16))
                wtp8 = wt_pairs[:, :, :].bitcast(F8)  # [128, C, 256]
                nb, off = divmod(nt, NPB)
                sl = slice(off * 128, (off + 1) * 128)
                src = wtp8.rearrange("p c (f two) -> p c two f", two=2)
                # deinterleave planes: i=0 on DVE, i=1 on ACT (balance)
                nc.vector.tensor_copy(wt_sep[:, :, nb, 0, sl], src[:, :, 0, :])
                nc.scalar.copy(wt_sep[:, :, nb, 1, sl], src[:, :, 1, :])

            # ---------------- amax of own tokens ----------------
            s_rev = cpool.tile([128, T], F32)
            for j in range(T):
                if j in pre_x:
                    xt = pre_x[j]  # kept alive: main loop rep 0 reuses
                else:
                    xt = spool.tile([128, K], F16, tag="xm", bufs=2,
                                    name=f"xm_{j}")
                    nc.sync.dma_start(xt[:, :], x_d[j * 128:(j + 1) * 128, :])
                xa = spool.tile([128, K], I16, tag="xa", bufs=2,
                                name=f"xa_{j}")
                nc.vector.tensor_scalar(xa[:, :], xt[:, :].bitcast(I16),
                                        0x7FFF, None, op0=AOP.bitwise_and)
                w = K // 2
                while w >= 512:
                    nc.vector.tensor_tensor(xa[:, :w], xa[:, :w],
                                            xa[:, w:2 * w], op=AOP.max)
                    w //= 2
                mbits = spool.tile([128, 1], I16, tag="mbits")
                nc.vector.tensor_reduce(mbits[:, :], xa[:, :2 * w],
                                        axis=mybir.AxisListType.X,
                                        op=AOP.max)
                nc.vector.tensor_scalar(s_rev[:, j:j + 1],
                                        mbits[:, :].bitcast(F16),
                                        1e-6, 1.0 / 7.0,
                                        op0=AOP.max, op1=AOP.mult)

            # reciprocal for quantization + flipped scales for the epilogue
            sq_all = cpool.tile([128, T], F32)
            nc.vector.reciprocal(sq_all[:, :], s_rev[:, :])
            ps_j = ppool.tile([128, T], F32, tag="psj", bufs=1)
            nc.tensor.matmul(ps_j[:, :], jm[:, :], s_rev[:, :],
                             start=True, stop=True)
            s_flip = cpool.tile([128, T], F32)
            nc.vector.tensor_copy(s_flip[:, :], ps_j[:, :])

            # ---------------- main loop ----------------
            for rep in range(repeat):
              for i in range(T):
                  if rep == 0 and i in pre_x:
                      xt = pre_x.pop(i)
                  else:
                      xt = xpool.tile([128, K], F16, tag="x", bufs=x_bufs,
                                      name=f"xt_{rep}_{i}")
                      nc.sync.dma_start(xt[:, :],
                                        x_d[i * 128:(i + 1) * 128, :])
                  # qb = fp16(x*sq + 1536): exact RNE integer round
                  nc.vector.tensor_scalar(xt[:, :], xt[:, :],
                                          sq_all[:, i:i + 1], 1536.0,
                                          op0=AOP.mult, op1=AOP.add)
                  # q8 = fp8(qb - 1536), alternating ACT/DVE
                  q8 = xpool.tile([128, K], F8, tag="q8", name=f"q8_{rep}_{i}")
                  if i % cast_mod != cast_mod - 1:
                      nc.scalar.activation(q8[:, :], xt[:, :], ACTF.Copy,
                                           bias=-1536.0, scale=1.0)
                  else:
                      nc.vector.tensor_scalar(q8[:, :], xt[:, :], 1536.0,
                                              None, op0=AOP.subtract)
                  # pair-transpose: qT[jj, c, f] = (q[f, 256c+2jj], +1)
                  qT = qpool.tile([128, C, 128], F16, tag="qT",
                                  name=f"qT_{rep}_{i}")
                  nc.scalar.dma_start_transpose(qT[:, :, :],
                                                q8[:, :].bitcast(F16))
                  qT8 = qT[:, :, :].bitcast(F8)  # [128, C, 256]

                  ps0 = ppool.tile([128, 1024], F32, tag="mm",
                                   name=f"ps0_{rep}_{i}")
                  ps1 = ppool.tile([128, 1024], F32, tag="mm",
                                   name=f"ps1_{rep}_{i}")
                  pss = (ps0, ps0, ps1, ps1)
                  for c in range(C):
                      for nb in range(NB):
                          nc.tensor.matmul(
                              pss[nb][:, (nb % 2) * 512:(nb % 2 + 1) * 512],
                              qT8[:, c, :],
                              wt_sep[:, c, nb, :, :],
                              start=(c == 0), stop=(c == C - 1),
                              perf_mode=SWI)
                  # epilogue: y = (ps * a_scale) * wscale + bias
                  t1 = epool.tile([128, NS], F16, tag="t1", name=f"t1_{rep}_{i}")
                  for h, ph in enumerate((ps0, ps1)):
                      nc.vector.scalar_tensor_tensor(
                          t1[:, h * 1024:(h + 1) * 1024], ph[:, :],
                          s_flip[:, i:i + 1],
                          wsc_bc[:, h * 1024:(h + 1) * 1024],
                          op0=AOP.mult, op1=AOP.mult)
                  nc.vector.tensor_tensor(t1[:, :], t1[:, :], bias_bc[:, :],
                                          op=AOP.add)
                  nc.scalar.dma_start(
                      y_d[MO - 128 * (i + 1):MO - 128 * i, :], t1[:, :])

    nc.compile()
    return nc


_CACHE = {}


def _get_nc():
    if "nc" not in _CACHE:
        _CACHE["nc"] = build()
    return _CACHE["nc"]


def _in_maps(x, qweight_packed, w_scales, bias):
    x2 = np.asarray(x).reshape(M, K)
    wsc = np.asarray(w_scales).reshape(N)
    bias = np.asarray(bias).reshape(N)
    in_maps = []
    for core in range(N_CORES):
        ti, ni = divmod(core, NSH)
        xsl = x2[ti * MO:(ti + 1) * MO]
        nsl = slice(ni * NS, (ni + 1) * NS)
        in_maps.append({
            "x": np.ascontiguousarray(xsl[::-1]),
            "wp": np.ascontiguousarray(np.asarray(qweight_packed)[nsl]),
            "wsc": np.ascontiguousarray(wsc[nsl]).reshape(1, NS),
            "bias": np.ascontiguousarray(bias[nsl]).reshape(1, NS),
        })
    return in_maps


def kernel(x, qweight_packed, w_scales, bias):
    nc = _get_nc()
    in_maps = _in_maps(x, qweight_packed, w_scales, bias)
    res = run_bass_kernel_spmd(nc, in_maps, core_ids=list(range(N_CORES)))
    y = np.empty((M, N), np.float16)
    for core in range(N_CORES):
        ti, ni = divmod(core, NSH)
        y[ti * MO:(ti + 1) * MO, ni * NS:(ni + 1) * NS] = res.results[core]["y"]
    return y.reshape(2, 2048, N)


# revision 4
# speedup vs baseline: 2.3639x; 1.0835x over previous
"""W4A4 quantized linear on 8 Trainium2 cores — v2: 4x token x 2x out-feature
sharding (no collectives), LDW-pipelined fp8 SwInterleave matmuls.

Per core: own 1024 tokens (row-reversed), 2048-row weight slice.
  y_blk = dequant_rowwise_quant(x_blk) @ dequant_w4(W_blk)^T + bias_blk

Device algorithm (per core), exact-integer math on the PE:
  setup: unpack W slice to fp8 k-planes (wt_sep[jj, c, nb, i, n]), amax of
         own tokens -> a_scales (+ J-matmul partition flip for the epilogue).
  per rep, per token tile i (8 tiles):
    qb = fp16(x * (1/a_scale) + 1536); q8 = fp8(qb - 1536)   (exact ints)
    qT = byte-pair DMA transpose (SwInterleave stationary format)
    for c in 16: for nb in 4: matmul(ps[:, nb], qT[c], wt_sep[c, nb], SWI)
    y = (ps * a_scale_flip) * wscale + bias  -> DMA out (row-reversed)

PE work: 8 tiles x 64 MMs x 512 cols = 109 us/core; everything else
(~21 MB DMA, ~25 us DVE, ~30 us ACT per rep) hides under it.
"""

import numpy as np
import concourse.bass as bass
import concourse.mybir as mybir
from concourse import bacc
from concourse.tile import TileContext
from concourse.bass_utils import run_bass_kernel_spmd

F8 = mybir.dt.float8e4
F16 = mybir.dt.float16
F32 = mybir.dt.float32
I8 = mybir.dt.int8
I16 = mybir.dt.int16
AOP = mybir.AluOpType
ACTF = mybir.ActivationFunctionType
SWI = mybir.MatmulPerfMode.DoubleRowSwInterleave

N_CORES = 8
TSH = 4            # token shards
NSH = 2            # out-feature shards
M, K, N = 4096, 4096, 4096
MO = M // TSH      # tokens per core (1024)
NS = N // NSH      # out features per core (2048)


def build(repeat=1, x_bufs=4, qt_bufs=3, cast_mod=2, mm_bufs=3,
          mm_only=False):
    T = MO // 128      # token tiles per core (8)
    NT = NS // 128     # weight row blocks (16)
    NB = NS // 512     # 512-col output blocks (4)
    C = K // 256       # contraction chunks (16)
    KP = K // 2        # packed weight columns
    NPB = NT // NB     # 128-row blocks per 512-col block (4)

    nc = bacc.Bacc("TRN2", target_bir_lowering=False, debug=False,
                   num_devices=N_CORES)

    x_d = nc.dram_tensor("x", [MO, K], F16, kind="ExternalInput")  # reversed!
    wp_d = nc.dram_tensor("wp", [NS, KP], I8, kind="ExternalInput")
    ws_d = nc.dram_tensor("wsc", [1, NS], F16, kind="ExternalInput")
    b_d = nc.dram_tensor("bias", [1, NS], F16, kind="ExternalInput")
    y_d = nc.dram_tensor("y", [MO, NS], F16, kind="ExternalOutput")

    with TileContext(nc) as tc:
        with (
            tc.tile_pool(name="const", bufs=1) as cpool,
            tc.tile_pool(name="wsetup", bufs=2) as wpool,
            tc.tile_pool(name="xwork", bufs=2) as xpool,
            tc.tile_pool(name="qtp", bufs=qt_bufs) as qpool,
            tc.tile_pool(name="small", bufs=3) as spool,
            tc.tile_pool(name="epi", bufs=2) as epool,
            tc.tile_pool(name="psum", bufs=mm_bufs, space="PSUM") as ppool,
        ):
            # ---------------- constants ----------------
            wsc_row = cpool.tile([1, NS], F16)
            nc.sync.dma_start(wsc_row[:, :], ws_d.ap())
            wsc_bc = cpool.tile([128, NS], F16)
            nc.gpsimd.partition_broadcast(wsc_bc[:, :], wsc_row[:, :])
            bias_row = cpool.tile([1, NS], F16)
            nc.sync.dma_start(bias_row[:, :], b_d.ap())
            bias_bc = cpool.tile([128, NS], F16)
            nc.gpsimd.partition_broadcast(bias_bc[:, :], bias_row[:, :])
            # anti-diagonal J for the partition flip
            jm = cpool.tile([128, 128], F32)
            nc.vector.memset(jm[:, :], 1.0)
            nc.gpsimd.affine_select(jm[:, :], jm[:, :], pattern=[[1, 128]],
                                    base=-127, channel_multiplier=1,
                                    compare_op=AOP.is_equal, fill=0.0)

            # ---------------- x prefetch (overlap with setup) ----------
            pre_x = {}
            for i in range(min(x_bufs, T)):
                xt = xpool.tile([128, K], F16, tag="x", bufs=x_bufs,
                                name=f"xt_0_{i}")
                nc.sync.dma_start(xt[:, :], x_d[i * 128:(i + 1) * 128, :])
                pre_x[i] = xt

            # ---------------- weight setup ----------------
            # wt_sep[jj, c, nb, i, n]: fp8 W[nb*512+n, 256c+2jj+i], each
            # (c, nb) slice is a contiguous [128, 2, 512] moving operand.
            wt_sep = cpool.tile([128, C, NB, 2, 512], F8)
            for nt in range(NT):
                wp_sb = wpool.tile([128, KP], I8, tag="wp")
                nc.sync.dma_start(wp_sb[:, :],
                                  wp_d[nt * 128:(nt + 1) * 128, :])
                w8 = wpool.tile([128, K], F8, tag="w8")
                w8v = w8[:, :].rearrange("p (j two) -> p j two", two=2)
                # high nibble = floor(b/16) (already sign-extended):
                # fp16(b/16 + 1535.53125) - 1536 via exact magic rounding
                hb = wpool.tile([128, KP], F16, tag="hb")
                nc.scalar.activation(hb[:, :], wp_sb[:, :], ACTF.Copy,
                                     bias=1535.53125, scale=1.0 / 16)
                nc.scalar.activation(w8v[:, :, 1], hb[:, :], ACTF.Copy,
                                     bias=-1536.0, scale=1.0)
                # low nibble: ((b & 15) ^ 8) - 8
                lo4 = wpool.tile([128, KP], I8, tag="lo4")
                nc.vector.tensor_scalar(lo4[:, :], wp_sb[:, :], 15, 8,
                                        op0=AOP.bitwise_and,
                                        op1=AOP.bitwise_xor)
                nc.vector.tensor_scalar(w8v[:, :, 0], lo4[:, :], 8.0, None,
                                        op0=AOP.subtract)
                # byte-pair transpose: [n-row, k] -> [k-pair jj, c, n]
                wt_pairs = wpool.tile([128, C, 128], F16, tag="wtp")
                nc.sync.dma_start_transpose(wt_pairs[:, :, :],
                                            w8[:, :].bitcast(F16))
                wtp8 = wt_pairs[:, :, :].bitcast(F8)  # [128, C, 256]
                nb, off = divmod(nt, NPB)
                sl = slice(off * 128, (off + 1) * 128)
                src = wtp8.rearrange("p c (f two) -> p c two f", two=2)
                # deinterleave planes: i=0 on DVE, i=1 on ACT (balance)
                nc.vector.tensor_copy(wt_sep[:, :, nb, 0, sl], src[:, :, 0, :])
                nc.scalar.copy(wt_sep[:, :, nb, 1, sl], src[:, :, 1, :])

            # ---------------- amax of own tokens ----------------
            s_rev = cpool.tile([128, T], F32)
            for j in range(T):
                if j in pre_x:
                    xt = pre_x[j]  # kept alive: main loop rep 0 reuses
                else:
                    xt = spool.tile([128, K], F16, tag="xm", bufs=2,
                                    name=f"xm_{j}")
                    nc.sync.dma_start(xt[:, :], x_d[j * 128:(j + 1) * 128, :])
                xa = spool.tile([128, K], I16, tag="xa", bufs=2,
                                name=f"xa_{j}")
                nc.vector.tensor_scalar(xa[:, :], xt[:, :].bitcast(I16),
                                        0x7FFF, None, op0=AOP.bitwise_and)
                w = K // 2
                while w >= 512:
                    nc.vector.tensor_tensor(xa[:, :w], xa[:, :w],
                                            xa[:, w:2 * w], op=AOP.max)
                    w //= 2
                mbits = spool.tile([128, 1], I16, tag="mbits")
                nc.vector.tensor_reduce(mbits[:, :], xa[:, :2 * w],
                                        axis=mybir.AxisListType.X,
                                        op=AOP.max)
                nc.vector.tensor_scalar(s_rev[:, j:j + 1],
                                        mbits[:, :].bitcast(F16),
                                        1e-6, 1.0 / 7.0,
                                        op0=AOP.max, op1=AOP.mult)

            # reciprocal for quantization + flipped scales for the epilogue
            sq_all = cpool.tile([128, T], F32)
            nc.vector.reciprocal(sq_all[:, :], s_rev[:, :])
            ps_j = ppool.tile([128, T], F32, tag="psj", bufs=1)
            nc.tensor.matmul(ps_j[:, :], jm[:, :], s_rev[:, :],
                             start=True, stop=True)
            s_flip = cpool.tile([128, T], F32)
            nc.vector.tensor_copy(s_flip[:, :], ps_j[:, :])

            # ---------------- main loop ----------------
            qTc = None
            if mm_only:  # diagnostic: constant stationary, no act pipeline
                qTc = cpool.tile([128, C, 128], F16)
                nc.vector.memset(qTc[:, :, :], 0.251)
            for rep in range(repeat):
              for i in range(T):
                  if mm_only:
                      qT = qTc
                  else:
                      if rep == 0 and i in pre_x:
                          xt = pre_x.pop(i)
                      else:
                          xt = xpool.tile([128, K], F16, tag="x", bufs=x_bufs,
                                          name=f"xt_{rep}_{i}")
                          nc.sync.dma_start(xt[:, :],
                                            x_d[i * 128:(i + 1) * 128, :])
                      # qb = fp16(x*sq + 1536): exact RNE integer round
                      nc.vector.tensor_scalar(xt[:, :], xt[:, :],
                                              sq_all[:, i:i + 1], 1536.0,
                                              op0=AOP.mult, op1=AOP.add)
                      # q8 = fp8(qb - 1536), alternating ACT/DVE
                      q8 = xpool.tile([128, K], F8, tag="q8",
                                      name=f"q8_{rep}_{i}")
                      if i % cast_mod != cast_mod - 1:
                          nc.scalar.activation(q8[:, :], xt[:, :], ACTF.Copy,
                                               bias=-1536.0, scale=1.0)
                      else:
                          nc.vector.tensor_scalar(q8[:, :], xt[:, :], 1536.0,
                                                  None, op0=AOP.subtract)
                      # pair-transpose: qT[jj, c, f] = (q[f, 256c+2jj], +1)
                      qT = qpool.tile([128, C, 128], F16, tag="qT",
                                      name=f"qT_{rep}_{i}")
                      nc.scalar.dma_start_transpose(qT[:, :, :],
                                                    q8[:, :].bitcast(F16))
                  qT8 = qT[:, :, :].bitcast(F8)  # [128, C, 256]

                  ps0 = ppool.tile([128, 1024], F32, tag="mm",
                                   name=f"ps0_{rep}_{i}")
                  ps1 = ppool.tile([128, 1024], F32, tag="mm",
                                   name=f"ps1_{rep}_{i}")
                  pss = (ps0, ps0, ps1, ps1)
                  for c in range(C):
                      for nb in range(NB):
                          nc.tensor.matmul(
                              pss[nb][:, (nb % 2) * 512:(nb % 2 + 1) * 512],
                              qT8[:, c, :],
                              wt_sep[:, c, nb, :, :],
                              start=(c == 0), stop=(c == C - 1),
                              perf_mode=SWI)
                  # epilogue: y = (ps * a_scale) * wscale + bias
                  t1 = epool.tile([128, NS], F16, tag="t1", name=f"t1_{rep}_{i}")
                  for h, ph in enumerate((ps0, ps1)):
                      nc.vector.scalar_tensor_tensor(
                          t1[:, h * 1024:(h + 1) * 1024], ph[:, :],
                          s_flip[:, i:i + 1],
                          wsc_bc[:, h * 1024:(h + 1) * 1024],
                          op0=AOP.mult, op1=AOP.mult)
                  nc.vector.tensor_tensor(t1[:, :], t1[:, :], bias_bc[:, :],
                                          op=AOP.add)
                  nc.scalar.dma_start(
                      y_d[MO - 128 * (i + 1):MO - 128 * i, :], t1[:, :])

    nc.compile()
    return nc


_CACHE = {}


def _get_nc():
    if "nc" not in _CACHE:
        _CACHE["nc"] = build()
    return _CACHE["nc"]


def _in_maps(x, qweight_packed, w_scales, bias):
    x2 = np.asarray(x).reshape(M, K)
    wsc = np.asarray(w_scales).reshape(N)
    bias = np.asarray(bias).reshape(N)
    in_maps = []
    for core in range(N_CORES):
        ti, ni = divmod(core, NSH)
        xsl = x2[ti * MO:(ti + 1) * MO]
        nsl = slice(ni * NS, (ni + 1) * NS)
        in_maps.append({
            "x": np.ascontiguousarray(xsl[::-1]),
            "wp": np.ascontiguousarray(np.asarray(qweight_packed)[nsl]),
            "wsc": np.ascontiguousarray(wsc[nsl]).reshape(1, NS),
            "bias": np.ascontiguousarray(bias[nsl]).reshape(1, NS),
        })
    return in_maps


def kernel(x, qweight_packed, w_scales, bias):
    nc = _get_nc()
    in_maps = _in_maps(x, qweight_packed, w_scales, bias)
    res = run_bass_kernel_spmd(nc, in_maps, core_ids=list(range(N_CORES)))
    y = np.empty((M, N), np.float16)
    for core in range(N_CORES):
        ti, ni = divmod(core, NSH)
        y[ti * MO:(ti + 1) * MO, ni * NS:(ni + 1) * NS] = res.results[core]["y"]
    return y.reshape(2, 2048, N)
